# revision 24
# baseline (speedup 1.0000x reference)
"""Trainium2 Bass kernel for per-variable gated LoRA mixer (dense_mlp).

Math (reference):
    xr  = x.reshape(b, t, v)                  # b=512, t=512, v=64
    x1  = tanh(gating * xr)
    tmp = einsum('biv,ik->bkv', x1, lora_A)   # r=16
    nx  = einsum('bkv,kov->bov', tmp, lora_B)
    out = xr + nx + bias

Split of work:
  HOST (numpy, fp32): pre-scale xs = gating*x, pre-transpose into the exact
    SBUF images (bf16), and the final out = x + nx + bias (nx from device).
  DEVICE (per core, 64 batch): x1 = tanh(xs) on ACT; mm1 (contract t) and
    mm2 (contract r, 4 vars packed block-diagonally per matmul) on PE in
    bf16; psum evacuation on ACT/DVE; all HBM I/O as large contiguous DMAs.

Device layouts (per core, b = 64 local batch):
  xs   [128, (g8=8, ch=4, j=4, vs=16, bl=8)] bf16: partition p = t within
       chunk ch (t = ch*128+p); g8*8+bl = local batch; v = 4*vs+j.
  a4   [128, (ch=4, 128)] bf16: 8 copies of lora_A chunk cols (16*cblk+k).
  mm1 per octet-pair P (g8 = 2P, 2P+1): psum p1 [128, (gp=2, 512)] fp32;
       rows 32j+k and 32j+16+k both hold tmp[k] (duplicated lhsT blocks)
       so the 32-row aligned copies p1[32j:32j+32] -> tmps[32j:32j+32] are
       legal (walrus requires 32-aligned compute-AP partition starts).
  tmps [128, (j=4, vs=16, g8=8, bl=8)] bf16: rows 32j..32j+15 = tmp k-rows
       of var class j, rows 32j+16..31 = duplicate junk (weights there = 0).
  lbw  [128, (q=4, g=16, o=128)] bf16: rows 32j+k = lora_B[k, q*128+o, 4g+j],
       rows 32j+16..31 = 0.
  mm2 per (q, g): lhsT = lbw slice [128, 128]; rhs = tmps AP [128, (j=4
       stride 1024, b=64)] at base g*64 -> psum [128 (o), (j, b) = 256] bf16,
       4 g's per psum bank; DVE copies bank -> out tile.
  out  [q=4][128, (g=16, j=4, b=64)] bf16 -> 1MB contiguous DMA per q.

Workarounds for this container's walrus build:
  - every instruction may carry at most ONE semaphore wait: TileContext's
    tail drain is patched and a post-pass hoists excess waits onto NoOps.
  - compute-engine APs must start at 32-aligned partitions.
"""

import numpy as np
import ml_dtypes

import concourse.bass as bass
import concourse.mybir as mybir
import concourse.tile as tile
from concourse.bass_utils import run_bass_kernel_spmd

F32 = mybir.dt.float32
BF16 = mybir.dt.bfloat16
F8 = mybir.dt.float8e4
BFNP = ml_dtypes.bfloat16
F8NP = ml_dtypes.float8_e4m3
XS_SCALE = 64.0     # xs shipped as fp8(XS_SCALE * gating * x); tanh rescales
NX_SCALE = 256.0    # lora_B pre-scaled so fp8 nx' = NX_SCALE * nx

N_CORES = 8
B_FULL = 512
T = 512          # window length (= o dim)
V = 64           # n_var
R = 16           # low rank
B = B_FULL // N_CORES   # 64 batch elements per core
NCH = T // 128   # 4 t-chunks
NG8 = B // 8     # 8 batch-octets
NP = 4           # octet-pairs in phase 1
J = 4            # var classes packed per mm2 matmul
VS = V // J      # 16 var slots per class
XCOLS = B * NCH * V          # 16384
QCOL = XCOLS // 4            # 4096 (one octet-pair of xs columns)


def _patch_tile_tail():
    """Re-emit the kernel-tail Drain's semaphore waits as individual
    wait_ge instructions (walrus here rejects multi-wait instructions)."""
    if getattr(tile.TileContext, "_drain_patched", False):
        return

    def _drain_and_barrier(self, tick_clock, wait_clock):
        nc = self.nc
        from concourse.tile import ScopedClock

        drain_inst = nc.sync.drain()
        wait_clock.add_sem_waits(
            drain_inst.ins, ScopedClock({None: tick_clock.global_clock})
        )
        si = drain_inst.ins.sync_info
        waits = list(si.on_wait) if si is not None else []
        if len(waits) > 1:
            sems_by_name = {s.name: s for s in self.sems.allocated().values()}
            si.on_wait = []
            for w in waits:
                nc.sync.wait_ge(sems_by_name[w.ant_name], w.wait_value)
        nc.all_engine_barrier()
        popped = nc._tile_sem_poison_stack.pop()
        assert popped is self._sem_poison
        nc.clear_and_free_semaphores(list(self.sems.allocated().values()))
        nc.all_engine_barrier()

    tile.TileContext._drain_and_barrier = _drain_and_barrier
    tile.TileContext._drain_patched = True


def _split_multi_waits(nc, limit=1):
    """Hoist excess semaphore waits onto same-engine NoOps inserted just
    before the offending instruction (program order per engine preserves
    the wait-before-execute semantics)."""
    ctr = 0
    for f in nc.m.functions:
        for b in f.blocks:
            insts = list(b.instructions)
            out = []
            changed = False
            for inst in insts:
                si = inst.sync_info
                if si is not None:
                    waits = list(si.on_wait)
                    if len(waits) > limit:
                        for w in waits[:-limit]:
                            nop = mybir.InstNoOp(name=f"zzws_{ctr}")
                            ctr += 1
                            nop.engine = inst.engine
                            nop.sync_info = mybir.SyncInfo(
                                on_wait=[w], on_update=[]
                            )
                            out.append(nop)
                        si.on_wait = waits[-limit:]
                        changed = True
                out.append(inst)
            if changed:
                b.instructions = out
    return ctr


def build_program():
    _patch_tile_tail()
    nc = bass.Bass()

    xs_d = nc.dram_tensor("xs", [128, XCOLS], F8, kind="ExternalInput")
    a4_d = nc.dram_tensor("a4", [128, NCH * 128], BF16, kind="ExternalInput")
    lbw_d = nc.dram_tensor("lbw", [128, NCH * VS * 128], BF16, kind="ExternalInput")
    out_d = nc.dram_tensor("out", [NCH, 128, VS * J * B], F8, kind="ExternalOutput")

    with tile.TileContext(nc) as tc:
        with (
            tc.tile_pool(name="pers", bufs=1) as pers,
            tc.tile_pool(name="outp", bufs=2) as outp,
            tc.tile_pool(name="ps1", bufs=2, space="PSUM") as ps1,
            tc.tile_pool(name="ps2", bufs=3, space="PSUM") as ps2,
            tc.tile_pool(name="psw", bufs=1, space="PSUM") as psw,
        ):
            xs = pers.tile([128, XCOLS], F8)             # 16KB/partition
            x1 = pers.tile([128, XCOLS], BF16)
            a4 = pers.tile([128, NCH * 128], BF16)
            lbw = pers.tile([128, NCH * VS * 128], BF16)
            tmps = pers.tile([128, J * VS * B], BF16)    # [128, 4096]
            warm = psw.tile([128, 512], F32)             # HAM warm-up target

            # --- input DMAs (xs quartered for overlap; lbw before last) ---
            nc.sync.dma_start(xs[:, 0:QCOL], xs_d[:, 0:QCOL])
            nc.sync.dma_start(a4[:, :], a4_d[:, :])
            for P in range(1, NP):
                nc.sync.dma_start(
                    xs[:, P * QCOL : (P + 1) * QCOL],
                    xs_d[:, P * QCOL : (P + 1) * QCOL],
                )
            nc.sync.dma_start(lbw[:, :], lbw_d[:, :])

            # block-diagonal rhs: off-class regions of tmps must be zero
            nc.vector.memset(tmps[:, :], 0.0)

            def warm_mms(n):
                # dummy matmuls: keep the PE's HAM activity window busy so
                # phase 2 runs at 2.4 GHz instead of the cold 1.2 GHz
                for _ in range(n):
                    nc.tensor.matmul(
                        warm[:, :], a4[:, 0:128], a4[:, :],
                        start=True, stop=True, skip_group_check=True,
                    )

            # --- phase 1: tanh + mm1 + tmp copies, per octet-pair P ---
            warm_mms(8)
            for P in range(NP):
                xq = xs[:, P * QCOL : (P + 1) * QCOL]
                x1q = x1[:, P * QCOL : (P + 1) * QCOL]
                nc.scalar.activation(
                    x1q, xq, mybir.ActivationFunctionType.Tanh,
                    scale=1.0 / XS_SCALE,
                )
                p1 = ps1.tile([128, 2 * 512], F32)       # 2 banks
                for gp in range(2):
                    g8 = 2 * P + gp
                    for ch in range(NCH):
                        nc.tensor.matmul(
                            p1[:, gp * 512 : (gp + 1) * 512],
                            a4[:, ch * 128 : (ch + 1) * 128],
                            x1[:, (g8 * NCH + ch) * 512 : (g8 * NCH + ch + 1) * 512],
                            start=(ch == 0),
                            stop=(ch == NCH - 1),
                        )
                # copies: rows 32j..32j+31 (duplicated tmp), class-j columns
                p1v = p1.rearrange("p (gp j vs b) -> p gp j vs b", gp=2, j=J, vs=VS)
                tv = tmps.rearrange("p (vs j g8 b) -> p g8 j vs b", vs=VS, j=J, g8=NG8)
                for j in range(J):
                    dst = tv[32 * j : 32 * j + 32, 2 * P : 2 * P + 2, j]
                    srcv = p1v[32 * j : 32 * j + 32, :, j]
                    if P == NP - 1 and j % 2 == 1:
                        nc.scalar.copy(dst, srcv)
                    else:
                        nc.vector.tensor_copy(dst, srcv)
                warm_mms(6 if P < NP - 1 else 14)

            # --- phase 2: mm2 + evac + out DMA, per o-chunk q ---
            tmpv = tmps.rearrange("p (g c) -> p g c", g=VS)
            for q in range(NCH):
                out_t = outp.tile([128, VS * J * B], F8)
                for g2 in range(8):
                    pb = ps2.tile([128, 2 * J * B], F32)    # 1 bank
                    for gg in range(2):
                        g = 2 * g2 + gg
                        nc.tensor.matmul(
                            pb[:, gg * 256 : (gg + 1) * 256],
                            lbw[:, (q * VS + g) * 128 : (q * VS + g + 1) * 128],
                            tmpv[:, g],
                            start=True,
                            stop=True,
                        )
                    # evac psum -> fp8 out tile, alternating DVE / ACT
                    dst = out_t[:, g2 * 512 : (g2 + 1) * 512]
                    if g2 % 2 == 0:
                        nc.vector.tensor_copy(dst, pb[:, :])
                    else:
                        nc.scalar.copy(dst, pb[:, :])
                nc.sync.dma_start(out_d[q], out_t[:, :])

    n_split = _split_multi_waits(nc)
    print(f"[kernel] wait-split nops inserted: {n_split}")
    return nc


_PROGRAM = None


def _get_program():
    global _PROGRAM
    if _PROGRAM is None:
        _PROGRAM = build_program()
    return _PROGRAM


def _host_prep(x, gating, lora_A, lora_B):
    # xs per core: [128, g8, ch, j, vs, bl] = gating[v] * x[b, t, v]
    # x: [512, 512, 64] -> [c, g8, bl, ch, p, vs, j]
    g2 = gating.reshape(VS, J)  # v = 4*vs + j
    xr = x.reshape(N_CORES, NG8, 8, NCH, 128, VS, J) * (XS_SCALE * g2)
    np.clip(xr, -240.0, 240.0, out=xr)
    xs_all = np.ascontiguousarray(
        xr.transpose(0, 4, 1, 3, 6, 5, 2).astype(F8NP)
    ).reshape(N_CORES, 128, XCOLS)

    # a4: [p, ch, 16*cblk + k] = lora_A[ch*128+p, k], 8 copies over cblk
    ap = lora_A.reshape(NCH, 128, R).transpose(1, 0, 2)  # [p, ch, k]
    a4 = np.ascontiguousarray(
        np.broadcast_to(ap[:, :, None, :], (128, NCH, 8, R)).astype(BFNP)
    ).reshape(128, NCH * 128)

    # lbw: rows 32j+k = B[k, q*128+o, 4g+j]; rows 32j+16.. = 0
    lb = (NX_SCALE * lora_B).reshape(R, NCH, 128, VS, J).transpose(4, 0, 1, 3, 2)  # [j,k,q,g,o]
    lbw = np.zeros((J, 32, NCH, VS, 128), dtype=BFNP)
    lbw[:, :R] = lb.astype(BFNP)
    lbw = lbw.reshape(128, NCH * VS * 128)
    return xs_all, a4, lbw


def kernel(x, gating, bias, lora_A, lora_B):
    x = np.ascontiguousarray(np.asarray(x, dtype=np.float32)).reshape(B_FULL, T, V)
    gating = np.asarray(gating, dtype=np.float32)
    bias = np.asarray(bias, dtype=np.float32)
    lora_A = np.ascontiguousarray(np.asarray(lora_A, dtype=np.float32))
    lora_B = np.ascontiguousarray(np.asarray(lora_B, dtype=np.float32))

    xs_all, a4, lbw = _host_prep(x, gating, lora_A, lora_B)

    nc = _get_program()
    in_maps = []
    for c in range(N_CORES):
        in_maps.append({"xs": xs_all[c], "a4": a4, "lbw": lbw})
    res = run_bass_kernel_spmd(nc, in_maps, core_ids=list(range(N_CORES)))

    out = np.empty((B_FULL, T, V), dtype=np.float32)
    xb = x.reshape(N_CORES, B, T, V)
    for c in range(N_CORES):
        # out_d [q, p, (g, j, b)] -> nx[b, (q, p), (vs=g, j)]
        nx = (
            res.results[c]["out"]
            .reshape(NCH, 128, VS, J, B)
            .transpose(4, 0, 1, 2, 3)
            .astype(np.float32)
            .reshape(B, T, V)
        )
        out[c * B : (c + 1) * B] = xb[c] + nx * (1.0 / NX_SCALE) + bias
    return out.reshape(B_FULL, T, V, 1)


# revision 27
# speedup vs baseline: 1.0351x; 1.0351x over previous
"""Trainium2 Bass kernel for per-variable gated LoRA mixer (dense_mlp).

Math (reference):
    xr  = x.reshape(b, t, v)                  # b=512, t=512, v=64
    x1  = tanh(gating * xr)
    tmp = einsum('biv,ik->bkv', x1, lora_A)   # r=16
    nx  = einsum('bkv,kov->bov', tmp, lora_B)
    out = xr + nx + bias

Split of work (data-parallel over batch, 64 per core, params replicated):
  HOST (numpy): scale/transpose inputs into matmul-ready SBUF images
    (xs = fp8(64*gating*x), lora_A copies, 256*lora_B in the block layout),
    and the final fp32 combine out = x + nx/256 + bias.
  DEVICE (per core): x1 = tanh(xs/64) on ACT (scale folded into the
    activation); mm1 (contract t) and mm2 (contract r) on the PE in
    bf16/fp8 with fp32 psum; psum evacuation split DVE/ACT; fp8 output.

Device layouts (b = 64 local batch, v = 4*vs + j):
  xs   [128, (g8=8, ch=4, j=4, vs=16, bl=8)] fp8: partition p = t within
       chunk ch (t = ch*128+p); local batch = g8*8+bl.
  a4   [128, (ch=4, 128)] bf16: 8 copies of the lora_A chunk per 16 cols,
       so mm1 psum rows 32j+k and 32j+16+k all hold tmp[k].
  mm1 per octet-pair P: psum p1 [128, (gp=2, 512)] fp32; copies
       p1[32j:32j+32] -> tmps[32j:32j+32] start at 32-aligned partitions
       (walrus requires 32-aligned compute-AP partition starts).
  tmps [128, (vs=16, j=4, g8=8, bl=8)] bf16: rows 32j..32j+15 hold tmp
       k-rows of var class j in its own column region, zero elsewhere
       (block-diagonal rhs); rows 32j+16..31 are duplicates that mm2
       multiplies by zero weight rows.
  lbw  [128, (q=4, g=16, o=128)] bf16: rows 32j+k = 256*lora_B[k, q*128+o,
       4g+j], rows 32j+16..31 = 0.
  mm2 per (q, g): lhsT = lbw slice [128, 128]; rhs = tmps [128, 256]
       contiguous -> psum [128 (o), (j, bl) = 256] fp32; 4 vars per matmul
       via the block-diagonal structure; DVE/ACT evacuate psum as fp8.
  out  [q=4][128, (g=16, j=4, b=64)] fp8 -> contiguous DMA per o-chunk q.

Workarounds for this container's walrus build:
  - every instruction may carry at most ONE semaphore wait: TileContext's
    tail drain is patched and a post-pass hoists excess waits onto NoOps.
  - compute-engine APs must start at 32-aligned partitions.
"""

import numpy as np
import ml_dtypes

import concourse.bass as bass
import concourse.mybir as mybir
import concourse.tile as tile
from concourse.bass_utils import run_bass_kernel_spmd

F32 = mybir.dt.float32
BF16 = mybir.dt.bfloat16
F8 = mybir.dt.float8e4
BFNP = ml_dtypes.bfloat16
F8NP = ml_dtypes.float8_e4m3
XS_SCALE = 64.0     # xs shipped as fp8(XS_SCALE * gating * x); tanh rescales
NX_SCALE = 256.0    # lora_B pre-scaled so fp8 nx' = NX_SCALE * nx

N_CORES = 8
B_FULL = 512
T = 512          # window length (= o dim)
V = 64           # n_var
R = 16           # low rank
B = B_FULL // N_CORES   # 64 batch elements per core
NCH = T // 128   # 4 t-chunks
NG8 = B // 8     # 8 batch-octets
NP = 4           # octet-pairs in phase 1
J = 4            # var classes packed per mm2 matmul
VS = V // J      # 16 var slots per class
XCOLS = B * NCH * V          # 16384
QCOL = XCOLS // 4            # 4096 (one octet-pair of xs columns)


def _patch_tile_tail():
    """Re-emit the kernel-tail Drain's semaphore waits as individual
    wait_ge instructions (walrus here rejects multi-wait instructions)."""
    if getattr(tile.TileContext, "_drain_patched", False):
        return

    def _drain_and_barrier(self, tick_clock, wait_clock):
        nc = self.nc
        from concourse.tile import ScopedClock

        drain_inst = nc.sync.drain()
        wait_clock.add_sem_waits(
            drain_inst.ins, ScopedClock({None: tick_clock.global_clock})
        )
        si = drain_inst.ins.sync_info
        waits = list(si.on_wait) if si is not None else []
        if len(waits) > 1:
            sems_by_name = {s.name: s for s in self.sems.allocated().values()}
            si.on_wait = []
            for w in waits:
                nc.sync.wait_ge(sems_by_name[w.ant_name], w.wait_value)
        nc.all_engine_barrier()
        popped = nc._tile_sem_poison_stack.pop()
        assert popped is self._sem_poison
        nc.clear_and_free_semaphores(list(self.sems.allocated().values()))
        nc.all_engine_barrier()

    tile.TileContext._drain_and_barrier = _drain_and_barrier
    tile.TileContext._drain_patched = True


def _split_multi_waits(nc, limit=1):
    """Hoist excess semaphore waits onto same-engine NoOps inserted just
    before the offending instruction (program order per engine preserves
    the wait-before-execute semantics)."""
    ctr = 0
    for f in nc.m.functions:
        for b in f.blocks:
            insts = list(b.instructions)
            out = []
            changed = False
            for inst in insts:
                si = inst.sync_info
                if si is not None:
                    waits = list(si.on_wait)
                    if len(waits) > limit:
                        for w in waits[:-limit]:
                            nop = mybir.InstNoOp(name=f"zzws_{ctr}")
                            ctr += 1
                            nop.engine = inst.engine
                            nop.sync_info = mybir.SyncInfo(
                                on_wait=[w], on_update=[]
                            )
                            out.append(nop)
                        si.on_wait = waits[-limit:]
                        changed = True
                out.append(inst)
            if changed:
                b.instructions = out
    return ctr


def build_program():
    _patch_tile_tail()
    nc = bass.Bass()

    xs_d = nc.dram_tensor("xs", [128, XCOLS], F8, kind="ExternalInput")
    a4_d = nc.dram_tensor("a4", [128, NCH * 128], BF16, kind="ExternalInput")
    lbw_d = nc.dram_tensor("lbw", [128, NCH * VS * 128], BF16, kind="ExternalInput")
    out_d = nc.dram_tensor("out", [NCH, 128, VS * J * B], F8, kind="ExternalOutput")

    with tile.TileContext(nc) as tc:
        with (
            tc.tile_pool(name="pers", bufs=1) as pers,
            tc.tile_pool(name="outp", bufs=2) as outp,
            tc.tile_pool(name="ps1", bufs=2, space="PSUM") as ps1,
            tc.tile_pool(name="ps2", bufs=3, space="PSUM") as ps2,
            tc.tile_pool(name="psw", bufs=1, space="PSUM") as psw,
        ):
            xs = pers.tile([128, XCOLS], F8)             # 16KB/partition
            x1 = pers.tile([128, XCOLS], BF16)
            a4 = pers.tile([128, NCH * 128], BF16)
            lbw = pers.tile([128, NCH * VS * 128], BF16)
            tmps = pers.tile([128, J * VS * B], BF16)    # [128, 4096]
            warm = psw.tile([128, 512], F32)             # HAM warm-up target
            junk = pers.tile([128, 512], BF16)           # never written: the
            # warm-up matmuls read it so they carry no input dependencies

            # --- input DMAs (xs quartered for overlap; first quarter split
            # into octets so the tanh chain starts as early as possible) ---
            nc.sync.dma_start(xs[:, 0:2048], xs_d[:, 0:2048])
            nc.sync.dma_start(xs[:, 2048:QCOL], xs_d[:, 2048:QCOL])
            nc.sync.dma_start(a4[:, :], a4_d[:, :])
            for P in range(1, NP):
                nc.sync.dma_start(
                    xs[:, P * QCOL : (P + 1) * QCOL],
                    xs_d[:, P * QCOL : (P + 1) * QCOL],
                )
            nc.sync.dma_start(lbw[:, :], lbw_d[:, :])

            # block-diagonal rhs: off-class regions of tmps must be zero
            nc.vector.memset(junk[:, :], 0.0)
            nc.vector.memset(tmps[:, :], 0.0)

            def warm_mms(n):
                # dummy matmuls: keep the PE's HAM activity window busy so
                # mm1/mm2 run at 2.4 GHz instead of the cold 1.2 GHz
                for _ in range(n):
                    nc.tensor.matmul(
                        warm[:, :], junk[:, 0:128], junk[:, :],
                        start=True, stop=True, skip_group_check=True,
                    )

            # --- phase 1: tanh + mm1 + tmp copies, per octet-pair P ---
            warm_mms(16)
            for P in range(NP):
                if P == 0:
                    for o8 in range(2):
                        nc.scalar.activation(
                            x1[:, o8 * 2048 : (o8 + 1) * 2048],
                            xs[:, o8 * 2048 : (o8 + 1) * 2048],
                            mybir.ActivationFunctionType.Tanh,
                            scale=1.0 / XS_SCALE,
                        )
                else:
                    nc.scalar.activation(
                        x1[:, P * QCOL : (P + 1) * QCOL],
                        xs[:, P * QCOL : (P + 1) * QCOL],
                        mybir.ActivationFunctionType.Tanh,
                        scale=1.0 / XS_SCALE,
                    )
                p1 = ps1.tile([128, 2 * 512], F32)       # 2 banks
                for gp in range(2):
                    g8 = 2 * P + gp
                    for ch in range(NCH):
                        nc.tensor.matmul(
                            p1[:, gp * 512 : (gp + 1) * 512],
                            a4[:, ch * 128 : (ch + 1) * 128],
                            x1[:, (g8 * NCH + ch) * 512 : (g8 * NCH + ch + 1) * 512],
                            start=(ch == 0),
                            stop=(ch == NCH - 1),
                        )
                # copies: rows 32j..32j+31 (duplicated tmp), class-j columns
                p1v = p1.rearrange("p (gp j vs b) -> p gp j vs b", gp=2, j=J, vs=VS)
                tv = tmps.rearrange("p (vs j g8 b) -> p g8 j vs b", vs=VS, j=J, g8=NG8)
                for j in range(J):
                    dst = tv[32 * j : 32 * j + 32, 2 * P : 2 * P + 2, j]
                    srcv = p1v[32 * j : 32 * j + 32, :, j]
                    if P == NP - 1 and j % 2 == 1:
                        nc.scalar.copy(dst, srcv)
                    else:
                        nc.vector.tensor_copy(dst, srcv)
                warm_mms(6 if P < NP - 1 else 14)

            # --- phase 2: mm2 + evac + out DMA, per o-chunk q ---
            tmpv = tmps.rearrange("p (g c) -> p g c", g=VS)
            for q in range(NCH):
                out_t = outp.tile([128, VS * J * B], F8)
                for g2 in range(8):
                    pb = ps2.tile([128, 2 * J * B], F32)    # 1 bank
                    for gg in range(2):
                        g = 2 * g2 + gg
                        nc.tensor.matmul(
                            pb[:, gg * 256 : (gg + 1) * 256],
                            lbw[:, (q * VS + g) * 128 : (q * VS + g + 1) * 128],
                            tmpv[:, g],
                            start=True,
                            stop=True,
                        )
                    # evac psum -> fp8 out tile, alternating DVE / ACT
                    dst = out_t[:, g2 * 512 : (g2 + 1) * 512]
                    if g2 % 2 == 0:
                        nc.vector.tensor_copy(dst, pb[:, :])
                    else:
                        nc.scalar.copy(dst, pb[:, :])
                nc.sync.dma_start(out_d[q], out_t[:, :])

    n_split = _split_multi_waits(nc)
    print(f"[kernel] wait-split nops inserted: {n_split}")
    return nc


_PROGRAM = None


def _get_program():
    global _PROGRAM
    if _PROGRAM is None:
        _PROGRAM = build_program()
    return _PROGRAM


def _host_prep(x, gating, lora_A, lora_B):
    # xs per core: [128, g8, ch, j, vs, bl] = gating[v] * x[b, t, v]
    # x: [512, 512, 64] -> [c, g8, bl, ch, p, vs, j]
    g2 = gating.reshape(VS, J)  # v = 4*vs + j
    xr = x.reshape(N_CORES, NG8, 8, NCH, 128, VS, J) * (XS_SCALE * g2)
    np.clip(xr, -240.0, 240.0, out=xr)
    xs_all = np.ascontiguousarray(
        xr.transpose(0, 4, 1, 3, 6, 5, 2).astype(F8NP)
    ).reshape(N_CORES, 128, XCOLS)

    # a4: [p, ch, 16*cblk + k] = lora_A[ch*128+p, k], 8 copies over cblk
    ap = lora_A.reshape(NCH, 128, R).transpose(1, 0, 2)  # [p, ch, k]
    a4 = np.ascontiguousarray(
        np.broadcast_to(ap[:, :, None, :], (128, NCH, 8, R)).astype(BFNP)
    ).reshape(128, NCH * 128)

    # lbw: rows 32j+k = B[k, q*128+o, 4g+j]; rows 32j+16.. = 0
    lb = (NX_SCALE * lora_B).reshape(R, NCH, 128, VS, J).transpose(4, 0, 1, 3, 2)  # [j,k,q,g,o]
    lbw = np.zeros((J, 32, NCH, VS, 128), dtype=BFNP)
    lbw[:, :R] = lb.astype(BFNP)
    lbw = lbw.reshape(128, NCH * VS * 128)
    return xs_all, a4, lbw


def kernel(x, gating, bias, lora_A, lora_B):
    x = np.ascontiguousarray(np.asarray(x, dtype=np.float32)).reshape(B_FULL, T, V)
    gating = np.asarray(gating, dtype=np.float32)
    bias = np.asarray(bias, dtype=np.float32)
    lora_A = np.ascontiguousarray(np.asarray(lora_A, dtype=np.float32))
    lora_B = np.ascontiguousarray(np.asarray(lora_B, dtype=np.float32))

    xs_all, a4, lbw = _host_prep(x, gating, lora_A, lora_B)

    nc = _get_program()
    in_maps = []
    for c in range(N_CORES):
        in_maps.append({"xs": xs_all[c], "a4": a4, "lbw": lbw})
    res = run_bass_kernel_spmd(nc, in_maps, core_ids=list(range(N_CORES)))

    out = np.empty((B_FULL, T, V), dtype=np.float32)
    xb = x.reshape(N_CORES, B, T, V)
    for c in range(N_CORES):
        # out_d [q, p, (g, j, b)] -> nx[b, (q, p), (vs=g, j)]
        nx = (
            res.results[c]["out"]
            .reshape(NCH, 128, VS, J, B)
            .transpose(4, 0, 1, 2, 3)
            .astype(np.float32)
            .reshape(B, T, V)
        )
        out[c * B : (c + 1) * B] = xb[c] + nx * (1.0 / NX_SCALE) + bias
    return out.reshape(B_FULL, T, V, 1)


# revision 28
# speedup vs baseline: 1.0366x; 1.0015x over previous
"""Trainium2 Bass kernel for per-variable gated LoRA mixer (dense_mlp).

Math (reference):
    xr  = x.reshape(b, t, v)                  # b=512, t=512, v=64
    x1  = tanh(gating * xr)
    tmp = einsum('biv,ik->bkv', x1, lora_A)   # r=16
    nx  = einsum('bkv,kov->bov', tmp, lora_B)
    out = xr + nx + bias

Split of work (data-parallel over batch, 64 per core, params replicated):
  HOST (numpy): scale/transpose inputs into matmul-ready SBUF images
    (xs = fp8(64*gating*x), lora_A copies, 256*lora_B in the block layout),
    and the final fp32 combine out = x + nx/256 + bias.
  DEVICE (per core): x1 = tanh(xs/64) on ACT (scale folded into the
    activation); mm1 (contract t) and mm2 (contract r) on the PE in
    bf16/fp8 with fp32 psum; psum evacuation split DVE/ACT; fp8 output.

Device layouts (b = 64 local batch, v = 4*vs + j):
  xs   [128, (g8=8, ch=4, j=4, vs=16, bl=8)] fp8: partition p = t within
       chunk ch (t = ch*128+p); local batch = g8*8+bl.
  a4   [128, (ch=4, 128)] bf16: 8 copies of the lora_A chunk per 16 cols,
       so mm1 psum rows 32j+k and 32j+16+k all hold tmp[k].
  mm1 per octet-pair P: psum p1 [128, (gp=2, 512)] fp32; copies
       p1[32j:32j+32] -> tmps[32j:32j+32] start at 32-aligned partitions
       (walrus requires 32-aligned compute-AP partition starts).
  tmps [128, (vs=16, j=4, g8=8, bl=8)] bf16: rows 32j..32j+15 hold tmp
       k-rows of var class j in its own column region, zero elsewhere
       (block-diagonal rhs); rows 32j+16..31 are duplicates that mm2
       multiplies by zero weight rows.
  lbw  [128, (q=4, g=16, o=128)] bf16: rows 32j+k = 256*lora_B[k, q*128+o,
       4g+j], rows 32j+16..31 = 0.
  mm2 per (q, g): lhsT = lbw slice [128, 128]; rhs = tmps [128, 256]
       contiguous -> psum [128 (o), (j, bl) = 256] fp32; 4 vars per matmul
       via the block-diagonal structure; DVE/ACT evacuate psum as fp8.
  out  [q=4][128, (g=16, j=4, b=64)] fp8 -> contiguous DMA per o-chunk q.

Workarounds for this container's walrus build:
  - every instruction may carry at most ONE semaphore wait: TileContext's
    tail drain is patched and a post-pass hoists excess waits onto NoOps.
  - compute-engine APs must start at 32-aligned partitions.
"""

import numpy as np
import ml_dtypes

import concourse.bass as bass
import concourse.mybir as mybir
import concourse.tile as tile
from concourse.bass_utils import run_bass_kernel_spmd

F32 = mybir.dt.float32
BF16 = mybir.dt.bfloat16
F8 = mybir.dt.float8e4
BFNP = ml_dtypes.bfloat16
F8NP = ml_dtypes.float8_e4m3
XS_SCALE = 64.0     # xs shipped as fp8(XS_SCALE * gating * x); tanh rescales
NX_SCALE = 256.0    # lora_B pre-scaled so fp8 nx' = NX_SCALE * nx

N_CORES = 8
B_FULL = 512
T = 512          # window length (= o dim)
V = 64           # n_var
R = 16           # low rank
B = B_FULL // N_CORES   # 64 batch elements per core
NCH = T // 128   # 4 t-chunks
NG8 = B // 8     # 8 batch-octets
NP = 4           # octet-pairs in phase 1
J = 4            # var classes packed per mm2 matmul
VS = V // J      # 16 var slots per class
XCOLS = B * NCH * V          # 16384
QCOL = XCOLS // 4            # 4096 (one octet-pair of xs columns)


def _patch_tile_tail():
    """Re-emit the kernel-tail Drain's semaphore waits as individual
    wait_ge instructions (walrus here rejects multi-wait instructions)."""
    if getattr(tile.TileContext, "_drain_patched", False):
        return

    def _drain_and_barrier(self, tick_clock, wait_clock):
        nc = self.nc
        from concourse.tile import ScopedClock

        drain_inst = nc.sync.drain()
        wait_clock.add_sem_waits(
            drain_inst.ins, ScopedClock({None: tick_clock.global_clock})
        )
        si = drain_inst.ins.sync_info
        waits = list(si.on_wait) if si is not None else []
        if len(waits) > 1:
            sems_by_name = {s.name: s for s in self.sems.allocated().values()}
            si.on_wait = []
            for w in waits:
                nc.sync.wait_ge(sems_by_name[w.ant_name], w.wait_value)
        nc.all_engine_barrier()
        popped = nc._tile_sem_poison_stack.pop()
        assert popped is self._sem_poison
        nc.clear_and_free_semaphores(list(self.sems.allocated().values()))
        nc.all_engine_barrier()

    tile.TileContext._drain_and_barrier = _drain_and_barrier
    tile.TileContext._drain_patched = True


def _split_multi_waits(nc, limit=1):
    """Hoist excess semaphore waits onto same-engine NoOps inserted just
    before the offending instruction (program order per engine preserves
    the wait-before-execute semantics)."""
    ctr = 0
    for f in nc.m.functions:
        for b in f.blocks:
            insts = list(b.instructions)
            out = []
            changed = False
            for inst in insts:
                si = inst.sync_info
                if si is not None:
                    waits = list(si.on_wait)
                    if len(waits) > limit:
                        for w in waits[:-limit]:
                            nop = mybir.InstNoOp(name=f"zzws_{ctr}")
                            ctr += 1
                            nop.engine = inst.engine
                            nop.sync_info = mybir.SyncInfo(
                                on_wait=[w], on_update=[]
                            )
                            out.append(nop)
                        si.on_wait = waits[-limit:]
                        changed = True
                out.append(inst)
            if changed:
                b.instructions = out
    return ctr


def build_program():
    _patch_tile_tail()
    nc = bass.Bass()

    xs_d = nc.dram_tensor("xs", [128, XCOLS], F8, kind="ExternalInput")
    a4_d = nc.dram_tensor("a4", [128, NCH * 128], BF16, kind="ExternalInput")
    lbw_d = nc.dram_tensor("lbw", [128, NCH * VS * 128], BF16, kind="ExternalInput")
    out_d = nc.dram_tensor("out", [NCH, 128, VS * J * B], F8, kind="ExternalOutput")

    with tile.TileContext(nc) as tc:
        with (
            tc.tile_pool(name="pers", bufs=1) as pers,
            tc.tile_pool(name="outp", bufs=2) as outp,
            tc.tile_pool(name="ps1", bufs=2, space="PSUM") as ps1,
            tc.tile_pool(name="ps2", bufs=3, space="PSUM") as ps2,
            tc.tile_pool(name="psw", bufs=1, space="PSUM") as psw,
        ):
            xs = pers.tile([128, XCOLS], F8)             # 16KB/partition
            x1 = pers.tile([128, XCOLS], BF16)
            a4 = pers.tile([128, NCH * 128], BF16)
            lbw = pers.tile([128, NCH * VS * 128], BF16)
            tmps = pers.tile([128, J * VS * B], BF16)    # [128, 4096]
            warm = psw.tile([128, 512], F32)             # HAM warm-up target
            junk = pers.tile([128, 512], BF16)           # never written: the
            # warm-up matmuls read it so they carry no input dependencies

            # --- input DMAs (xs quartered for overlap; first quarter split
            # into octets so the tanh chain starts as early as possible) ---
            nc.sync.dma_start(xs[:, 0:2048], xs_d[:, 0:2048])
            nc.sync.dma_start(xs[:, 2048:QCOL], xs_d[:, 2048:QCOL])
            nc.sync.dma_start(a4[:, :], a4_d[:, :])
            for P in range(1, NP):
                nc.sync.dma_start(
                    xs[:, P * QCOL : (P + 1) * QCOL],
                    xs_d[:, P * QCOL : (P + 1) * QCOL],
                )
            nc.sync.dma_start(lbw[:, :], lbw_d[:, :])

            # block-diagonal rhs: off-class regions of tmps must be zero
            nc.vector.memset(junk[:, :], 0.0)
            nc.vector.memset(tmps[:, :], 0.0)

            def warm_mms(n):
                # dummy matmuls: keep the PE's HAM activity window busy so
                # mm1/mm2 run at 2.4 GHz instead of the cold 1.2 GHz
                for _ in range(n):
                    nc.tensor.matmul(
                        warm[:, :], junk[:, 0:128], junk[:, :],
                        start=True, stop=True, skip_group_check=True,
                    )

            # --- phase 1: tanh + mm1 + tmp copies, per octet-pair P ---
            warm_mms(16)
            tv = tmps.rearrange("p (vs j g8 b) -> p g8 j vs b", vs=VS, j=J, g8=NG8)
            for P in range(NP):
                octet_split = P in (0, NP - 1)
                if octet_split:
                    # first pair: tanh starts on the smallest possible DMA;
                    # last pair: mm1/copies of octet 0 hide under octet 1's
                    # tanh so phase 2 starts sooner
                    for gp in range(2):
                        o8 = 2 * P + gp
                        nc.scalar.activation(
                            x1[:, o8 * 2048 : (o8 + 1) * 2048],
                            xs[:, o8 * 2048 : (o8 + 1) * 2048],
                            mybir.ActivationFunctionType.Tanh,
                            scale=1.0 / XS_SCALE,
                        )
                else:
                    nc.scalar.activation(
                        x1[:, P * QCOL : (P + 1) * QCOL],
                        xs[:, P * QCOL : (P + 1) * QCOL],
                        mybir.ActivationFunctionType.Tanh,
                        scale=1.0 / XS_SCALE,
                    )
                p1 = ps1.tile([128, 2 * 512], F32)       # 2 banks
                p1v = p1.rearrange("p (gp j vs b) -> p gp j vs b", gp=2, j=J, vs=VS)
                for gp in range(2):
                    g8 = 2 * P + gp
                    for ch in range(NCH):
                        nc.tensor.matmul(
                            p1[:, gp * 512 : (gp + 1) * 512],
                            a4[:, ch * 128 : (ch + 1) * 128],
                            x1[:, (g8 * NCH + ch) * 512 : (g8 * NCH + ch + 1) * 512],
                            start=(ch == 0),
                            stop=(ch == NCH - 1),
                        )
                    if P == NP - 1:
                        # per-octet copies (different psum bank than the
                        # in-flight gp=1 matmuls)
                        for j in range(J):
                            dst = tv[32 * j : 32 * j + 32, g8 : g8 + 1, j]
                            srcv = p1v[32 * j : 32 * j + 32, gp : gp + 1, j]
                            if gp == 1 and j % 2 == 1:
                                nc.scalar.copy(dst, srcv)
                            else:
                                nc.vector.tensor_copy(dst, srcv)
                if P < NP - 1:
                    for j in range(J):
                        nc.vector.tensor_copy(
                            tv[32 * j : 32 * j + 32, 2 * P : 2 * P + 2, j],
                            p1v[32 * j : 32 * j + 32, :, j],
                        )
                warm_mms(6 if P < NP - 1 else 14)

            # --- phase 2: mm2 + evac + out DMA, per o-chunk q ---
            tmpv = tmps.rearrange("p (g c) -> p g c", g=VS)
            for q in range(NCH):
                out_t = outp.tile([128, VS * J * B], F8)
                for g2 in range(8):
                    pb = ps2.tile([128, 2 * J * B], F32)    # 1 bank
                    for gg in range(2):
                        g = 2 * g2 + gg
                        nc.tensor.matmul(
                            pb[:, gg * 256 : (gg + 1) * 256],
                            lbw[:, (q * VS + g) * 128 : (q * VS + g + 1) * 128],
                            tmpv[:, g],
                            start=True,
                            stop=True,
                        )
                    # evac psum -> fp8 out tile, alternating DVE / ACT
                    dst = out_t[:, g2 * 512 : (g2 + 1) * 512]
                    if g2 % 2 == 0:
                        nc.vector.tensor_copy(dst, pb[:, :])
                    else:
                        nc.scalar.copy(dst, pb[:, :])
                    if g2 == 3:
                        nc.sync.dma_start(
                            out_d[q, :, 0:2048], out_t[:, 0:2048]
                        )
                nc.sync.dma_start(out_d[q, :, 2048:], out_t[:, 2048:])

    n_split = _split_multi_waits(nc)
    print(f"[kernel] wait-split nops inserted: {n_split}")
    return nc


_PROGRAM = None


def _get_program():
    global _PROGRAM
    if _PROGRAM is None:
        _PROGRAM = build_program()
    return _PROGRAM


def _host_prep(x, gating, lora_A, lora_B):
    # xs per core: [128, g8, ch, j, vs, bl] = gating[v] * x[b, t, v]
    # x: [512, 512, 64] -> [c, g8, bl, ch, p, vs, j]
    g2 = gating.reshape(VS, J)  # v = 4*vs + j
    xr = x.reshape(N_CORES, NG8, 8, NCH, 128, VS, J) * (XS_SCALE * g2)
    np.clip(xr, -240.0, 240.0, out=xr)
    xs_all = np.ascontiguousarray(
        xr.transpose(0, 4, 1, 3, 6, 5, 2).astype(F8NP)
    ).reshape(N_CORES, 128, XCOLS)

    # a4: [p, ch, 16*cblk + k] = lora_A[ch*128+p, k], 8 copies over cblk
    ap = lora_A.reshape(NCH, 128, R).transpose(1, 0, 2)  # [p, ch, k]
    a4 = np.ascontiguousarray(
        np.broadcast_to(ap[:, :, None, :], (128, NCH, 8, R)).astype(BFNP)
    ).reshape(128, NCH * 128)

    # lbw: rows 32j+k = B[k, q*128+o, 4g+j]; rows 32j+16.. = 0
    lb = (NX_SCALE * lora_B).reshape(R, NCH, 128, VS, J).transpose(4, 0, 1, 3, 2)  # [j,k,q,g,o]
    lbw = np.zeros((J, 32, NCH, VS, 128), dtype=BFNP)
    lbw[:, :R] = lb.astype(BFNP)
    lbw = lbw.reshape(128, NCH * VS * 128)
    return xs_all, a4, lbw


def kernel(x, gating, bias, lora_A, lora_B):
    x = np.ascontiguousarray(np.asarray(x, dtype=np.float32)).reshape(B_FULL, T, V)
    gating = np.asarray(gating, dtype=np.float32)
    bias = np.asarray(bias, dtype=np.float32)
    lora_A = np.ascontiguousarray(np.asarray(lora_A, dtype=np.float32))
    lora_B = np.ascontiguousarray(np.asarray(lora_B, dtype=np.float32))

    xs_all, a4, lbw = _host_prep(x, gating, lora_A, lora_B)

    nc = _get_program()
    in_maps = []
    for c in range(N_CORES):
        in_maps.append({"xs": xs_all[c], "a4": a4, "lbw": lbw})
    res = run_bass_kernel_spmd(nc, in_maps, core_ids=list(range(N_CORES)))

    out = np.empty((B_FULL, T, V), dtype=np.float32)
    xb = x.reshape(N_CORES, B, T, V)
    for c in range(N_CORES):
        # out_d [q, p, (g, j, b)] -> nx[b, (q, p), (vs=g, j)]
        nx = (
            res.results[c]["out"]
            .reshape(NCH, 128, VS, J, B)
            .transpose(4, 0, 1, 2, 3)
            .astype(np.float32)
            .reshape(B, T, V)
        )
        out[c * B : (c + 1) * B] = xb[c] + nx * (1.0 / NX_SCALE) + bias
    return out.reshape(B_FULL, T, V, 1)


# revision 29
# speedup vs baseline: 1.0654x; 1.0278x over previous
"""Trainium2 Bass kernel for per-variable gated LoRA mixer (dense_mlp).

Math (reference):
    xr  = x.reshape(b, t, v)                  # b=512, t=512, v=64
    x1  = tanh(gating * xr)
    tmp = einsum('biv,ik->bkv', x1, lora_A)   # r=16
    nx  = einsum('bkv,kov->bov', tmp, lora_B)
    out = xr + nx + bias

Split of work (data-parallel over batch, 64 per core, params replicated):
  HOST (numpy): scale/transpose inputs into matmul-ready SBUF images
    (xs = fp8(64*gating*x), lora_A copies, 256*lora_B in the block layout),
    and the final fp32 combine out = x + nx/256 + bias.
  DEVICE (per core): x1 = tanh(xs/64) on ACT (scale folded into the
    activation); mm1 (contract t) and mm2 (contract r) on the PE in
    bf16/fp8 with fp32 psum; psum evacuation split DVE/ACT; fp8 output.

Device layouts (b = 64 local batch, v = 4*vs + j):
  xs   [128, (g8=8, ch=4, j=4, vs=16, bl=8)] fp8: partition p = t within
       chunk ch (t = ch*128+p); local batch = g8*8+bl.
  a4   [128, (ch=4, 128)] bf16: 8 copies of the lora_A chunk per 16 cols,
       so mm1 psum rows 32j+k and 32j+16+k all hold tmp[k].
  mm1 per octet-pair P: psum p1 [128, (gp=2, 512)] fp32; copies
       p1[32j:32j+32] -> tmps[32j:32j+32] start at 32-aligned partitions
       (walrus requires 32-aligned compute-AP partition starts).
  tmps [128, (vs=16, j=4, g8=8, bl=8)] bf16: rows 32j..32j+15 hold tmp
       k-rows of var class j in its own column region, zero elsewhere
       (block-diagonal rhs); rows 32j+16..31 are duplicates that mm2
       multiplies by zero weight rows.
  lbw  [128, (q=4, g=16, o=128)] bf16: rows 32j+k = 256*lora_B[k, q*128+o,
       4g+j], rows 32j+16..31 = 0.
  mm2 per (q, g): lhsT = lbw slice [128, 128]; rhs = tmps [128, 256]
       contiguous -> psum [128 (o), (j, bl) = 256] fp32; 4 vars per matmul
       via the block-diagonal structure; DVE/ACT evacuate psum as fp8.
  out  [q=4][128, (g=16, j=4, b=64)] fp8 -> contiguous DMA per o-chunk q.

Workarounds for this container's walrus build:
  - every instruction may carry at most ONE semaphore wait: TileContext's
    tail drain is patched and a post-pass hoists excess waits onto NoOps.
  - compute-engine APs must start at 32-aligned partitions.
"""

import numpy as np
import ml_dtypes

import concourse.bass as bass
import concourse.mybir as mybir
import concourse.tile as tile
from concourse.bass_utils import run_bass_kernel_spmd

F32 = mybir.dt.float32
BF16 = mybir.dt.bfloat16
F8 = mybir.dt.float8e4
BFNP = ml_dtypes.bfloat16
F8NP = ml_dtypes.float8_e4m3
XS_SCALE = 64.0     # xs shipped as fp8(XS_SCALE * gating * x); tanh rescales
NX_SCALE = 256.0    # lora_B pre-scaled so fp8 nx' = NX_SCALE * nx

N_CORES = 8
B_FULL = 512
T = 512          # window length (= o dim)
V = 64           # n_var
R = 16           # low rank
B = B_FULL // N_CORES   # 64 batch elements per core
NCH = T // 128   # 4 t-chunks
NG8 = B // 8     # 8 batch-octets
NP = 4           # octet-pairs in phase 1
J = 4            # var classes packed per mm2 matmul
VS = V // J      # 16 var slots per class
XCOLS = B * NCH * V          # 16384
QCOL = XCOLS // 4            # 4096 (one octet-pair of xs columns)


def _patch_tile_tail():
    """Re-emit the kernel-tail Drain's semaphore waits as individual
    wait_ge instructions (walrus here rejects multi-wait instructions)."""
    if getattr(tile.TileContext, "_drain_patched", False):
        return

    def _drain_and_barrier(self, tick_clock, wait_clock):
        nc = self.nc
        from concourse.tile import ScopedClock

        drain_inst = nc.sync.drain()
        wait_clock.add_sem_waits(
            drain_inst.ins, ScopedClock({None: tick_clock.global_clock})
        )
        si = drain_inst.ins.sync_info
        waits = list(si.on_wait) if si is not None else []
        if len(waits) > 1:
            sems_by_name = {s.name: s for s in self.sems.allocated().values()}
            si.on_wait = []
            for w in waits:
                nc.sync.wait_ge(sems_by_name[w.ant_name], w.wait_value)
        nc.all_engine_barrier()
        popped = nc._tile_sem_poison_stack.pop()
        assert popped is self._sem_poison
        nc.clear_and_free_semaphores(list(self.sems.allocated().values()))
        nc.all_engine_barrier()

    tile.TileContext._drain_and_barrier = _drain_and_barrier
    tile.TileContext._drain_patched = True


def _split_multi_waits(nc, limit=1):
    """Hoist excess semaphore waits onto same-engine NoOps inserted just
    before the offending instruction (program order per engine preserves
    the wait-before-execute semantics)."""
    ctr = 0
    for f in nc.m.functions:
        for b in f.blocks:
            insts = list(b.instructions)
            out = []
            changed = False
            for inst in insts:
                si = inst.sync_info
                if si is not None:
                    waits = list(si.on_wait)
                    if len(waits) > limit:
                        for w in waits[:-limit]:
                            nop = mybir.InstNoOp(name=f"zzws_{ctr}")
                            ctr += 1
                            nop.engine = inst.engine
                            nop.sync_info = mybir.SyncInfo(
                                on_wait=[w], on_update=[]
                            )
                            out.append(nop)
                        si.on_wait = waits[-limit:]
                        changed = True
                out.append(inst)
            if changed:
                b.instructions = out
    return ctr


def build_program():
    _patch_tile_tail()
    nc = bass.Bass()

    xs_d = nc.dram_tensor("xs", [128, XCOLS], F8, kind="ExternalInput")
    a4_d = nc.dram_tensor("a4", [128, NCH * 128], BF16, kind="ExternalInput")
    lbw_d = nc.dram_tensor("lbw", [128, NCH * VS * 128], BF16, kind="ExternalInput")
    out_d = nc.dram_tensor("out", [NCH, 128, VS * J * B], F8, kind="ExternalOutput")

    with tile.TileContext(nc) as tc:
        with (
            tc.tile_pool(name="pers", bufs=1) as pers,
            tc.tile_pool(name="outp", bufs=2) as outp,
            tc.tile_pool(name="ps1", bufs=2, space="PSUM") as ps1,
            tc.tile_pool(name="ps2", bufs=3, space="PSUM") as ps2,
            tc.tile_pool(name="psw", bufs=1, space="PSUM") as psw,
        ):
            xs = pers.tile([128, XCOLS], F8)             # 16KB/partition
            x1 = pers.tile([128, XCOLS], BF16)
            a4 = pers.tile([128, NCH * 128], BF16)
            lbw = pers.tile([128, NCH * VS * 128], BF16)
            tmps = pers.tile([128, J * VS * B], BF16)    # [128, 4096]
            warm = psw.tile([128, 512], F32)             # HAM warm-up target
            junk = pers.tile([128, 512], BF16)           # never written: the
            # warm-up matmuls read it so they carry no input dependencies

            # --- input DMAs (xs quartered for overlap; first quarter split
            # into octets so the tanh chain starts as early as possible) ---
            nc.sync.dma_start(xs[:, 0:2048], xs_d[:, 0:2048])
            nc.sync.dma_start(xs[:, 2048:QCOL], xs_d[:, 2048:QCOL])
            nc.sync.dma_start(a4[:, :], a4_d[:, :])
            for P in range(1, NP):
                nc.sync.dma_start(
                    xs[:, P * QCOL : (P + 1) * QCOL],
                    xs_d[:, P * QCOL : (P + 1) * QCOL],
                )
            nc.sync.dma_start(lbw[:, :], lbw_d[:, :])

            # block-diagonal rhs: off-class regions of tmps must be zero
            nc.vector.memset(junk[:, :], 0.0)
            nc.vector.memset(tmps[:, :], 0.0)

            def warm_mms(n):
                # dummy matmuls: keep the PE's HAM activity window busy so
                # mm1/mm2 run at 2.4 GHz instead of the cold 1.2 GHz
                for _ in range(n):
                    nc.tensor.matmul(
                        warm[:, :], junk[:, 0:128], junk[:, :],
                        start=True, stop=True, skip_group_check=True,
                    )

            # --- phase 1: tanh + mm1 + tmp copies, per octet-pair P ---
            warm_mms(16)
            tv = tmps.rearrange("p (vs j g8 b) -> p g8 j vs b", vs=VS, j=J, g8=NG8)
            for P in range(NP):
                octet_split = P in (0, NP - 1)
                if octet_split:
                    # first pair: tanh starts on the smallest possible DMA;
                    # last pair: mm1/copies of octet 0 hide under octet 1's
                    # tanh so phase 2 starts sooner
                    for gp in range(2):
                        o8 = 2 * P + gp
                        nc.scalar.activation(
                            x1[:, o8 * 2048 : (o8 + 1) * 2048],
                            xs[:, o8 * 2048 : (o8 + 1) * 2048],
                            mybir.ActivationFunctionType.Tanh,
                            scale=1.0 / XS_SCALE,
                        )
                else:
                    nc.scalar.activation(
                        x1[:, P * QCOL : (P + 1) * QCOL],
                        xs[:, P * QCOL : (P + 1) * QCOL],
                        mybir.ActivationFunctionType.Tanh,
                        scale=1.0 / XS_SCALE,
                    )
                p1 = ps1.tile([128, 2 * 512], F32)       # 2 banks
                p1v = p1.rearrange("p (gp j vs b) -> p gp j vs b", gp=2, j=J, vs=VS)
                for gp in range(2):
                    g8 = 2 * P + gp
                    for ch in range(NCH):
                        nc.tensor.matmul(
                            p1[:, gp * 512 : (gp + 1) * 512],
                            a4[:, ch * 128 : (ch + 1) * 128],
                            x1[:, (g8 * NCH + ch) * 512 : (g8 * NCH + ch + 1) * 512],
                            start=(ch == 0),
                            stop=(ch == NCH - 1),
                        )
                    if P == NP - 1:
                        # per-octet copies (different psum bank than the
                        # in-flight gp=1 matmuls)
                        for j in range(J):
                            dst = tv[32 * j : 32 * j + 32, g8 : g8 + 1, j]
                            srcv = p1v[32 * j : 32 * j + 32, gp : gp + 1, j]
                            if gp == 1 and j % 2 == 1:
                                nc.scalar.copy(dst, srcv)
                            else:
                                nc.vector.tensor_copy(dst, srcv)
                if P < NP - 1:
                    for j in range(J):
                        nc.vector.tensor_copy(
                            tv[32 * j : 32 * j + 32, 2 * P : 2 * P + 2, j],
                            p1v[32 * j : 32 * j + 32, :, j],
                        )
                warm_mms(2 if P < NP - 1 else 10)

            # --- phase 2: mm2 + evac + out DMA, per o-chunk q ---
            tmpv = tmps.rearrange("p (g c) -> p g c", g=VS)
            for q in range(NCH):
                out_t = outp.tile([128, VS * J * B], F8)
                for g2 in range(8):
                    pb = ps2.tile([128, 2 * J * B], F32)    # 1 bank
                    for gg in range(2):
                        g = 2 * g2 + gg
                        nc.tensor.matmul(
                            pb[:, gg * 256 : (gg + 1) * 256],
                            lbw[:, (q * VS + g) * 128 : (q * VS + g + 1) * 128],
                            tmpv[:, g],
                            start=True,
                            stop=True,
                        )
                    # evac psum -> fp8 out tile, alternating DVE / ACT
                    dst = out_t[:, g2 * 512 : (g2 + 1) * 512]
                    if g2 % 2 == 0:
                        nc.vector.tensor_copy(dst, pb[:, :])
                    else:
                        nc.scalar.copy(dst, pb[:, :])
                    if g2 == 3:
                        nc.sync.dma_start(
                            out_d[q, :, 0:2048], out_t[:, 0:2048]
                        )
                nc.sync.dma_start(out_d[q, :, 2048:], out_t[:, 2048:])

    n_split = _split_multi_waits(nc)
    print(f"[kernel] wait-split nops inserted: {n_split}")
    return nc


_PROGRAM = None


def _get_program():
    global _PROGRAM
    if _PROGRAM is None:
        _PROGRAM = build_program()
    return _PROGRAM


def _host_prep(x, gating, lora_A, lora_B):
    # xs per core: [128, g8, ch, j, vs, bl] = gating[v] * x[b, t, v]
    # x: [512, 512, 64] -> [c, g8, bl, ch, p, vs, j]
    g2 = gating.reshape(VS, J)  # v = 4*vs + j
    xr = x.reshape(N_CORES, NG8, 8, NCH, 128, VS, J) * (XS_SCALE * g2)
    np.clip(xr, -240.0, 240.0, out=xr)
    xs_all = np.ascontiguousarray(
        xr.transpose(0, 4, 1, 3, 6, 5, 2).astype(F8NP)
    ).reshape(N_CORES, 128, XCOLS)

    # a4: [p, ch, 16*cblk + k] = lora_A[ch*128+p, k], 8 copies over cblk
    ap = lora_A.reshape(NCH, 128, R).transpose(1, 0, 2)  # [p, ch, k]
    a4 = np.ascontiguousarray(
        np.broadcast_to(ap[:, :, None, :], (128, NCH, 8, R)).astype(BFNP)
    ).reshape(128, NCH * 128)

    # lbw: rows 32j+k = B[k, q*128+o, 4g+j]; rows 32j+16.. = 0
    lb = (NX_SCALE * lora_B).reshape(R, NCH, 128, VS, J).transpose(4, 0, 1, 3, 2)  # [j,k,q,g,o]
    lbw = np.zeros((J, 32, NCH, VS, 128), dtype=BFNP)
    lbw[:, :R] = lb.astype(BFNP)
    lbw = lbw.reshape(128, NCH * VS * 128)
    return xs_all, a4, lbw


def kernel(x, gating, bias, lora_A, lora_B):
    x = np.ascontiguousarray(np.asarray(x, dtype=np.float32)).reshape(B_FULL, T, V)
    gating = np.asarray(gating, dtype=np.float32)
    bias = np.asarray(bias, dtype=np.float32)
    lora_A = np.ascontiguousarray(np.asarray(lora_A, dtype=np.float32))
    lora_B = np.ascontiguousarray(np.asarray(lora_B, dtype=np.float32))

    xs_all, a4, lbw = _host_prep(x, gating, lora_A, lora_B)

    nc = _get_program()
    in_maps = []
    for c in range(N_CORES):
        in_maps.append({"xs": xs_all[c], "a4": a4, "lbw": lbw})
    res = run_bass_kernel_spmd(nc, in_maps, core_ids=list(range(N_CORES)))

    out = np.empty((B_FULL, T, V), dtype=np.float32)
    xb = x.reshape(N_CORES, B, T, V)
    for c in range(N_CORES):
        # out_d [q, p, (g, j, b)] -> nx[b, (q, p), (vs=g, j)]
        nx = (
            res.results[c]["out"]
            .reshape(NCH, 128, VS, J, B)
            .transpose(4, 0, 1, 2, 3)
            .astype(np.float32)
            .reshape(B, T, V)
        )
        out[c * B : (c + 1) * B] = xb[c] + nx * (1.0 / NX_SCALE) + bias
    return out.reshape(B_FULL, T, V, 1)


# revision 30
# speedup vs baseline: 1.0777x; 1.0116x over previous
"""Trainium2 Bass kernel for per-variable gated LoRA mixer (dense_mlp).

Math (reference):
    xr  = x.reshape(b, t, v)                  # b=512, t=512, v=64
    x1  = tanh(gating * xr)
    tmp = einsum('biv,ik->bkv', x1, lora_A)   # r=16
    nx  = einsum('bkv,kov->bov', tmp, lora_B)
    out = xr + nx + bias

Split of work (data-parallel over batch, 64 per core, params replicated):
  HOST (numpy): scale/transpose inputs into matmul-ready SBUF images
    (xs = fp8(64*gating*x), lora_A copies, 256*lora_B in the block layout),
    and the final fp32 combine out = x + nx/256 + bias.
  DEVICE (per core): x1 = tanh(xs/64) on ACT (scale folded into the
    activation); mm1 (contract t) and mm2 (contract r) on the PE in
    bf16/fp8 with fp32 psum; psum evacuation split DVE/ACT; fp8 output.

Device layouts (b = 64 local batch, v = 4*vs + j):
  xs   [128, (g8=8, ch=4, j=4, vs=16, bl=8)] fp8: partition p = t within
       chunk ch (t = ch*128+p); local batch = g8*8+bl.
  a4   [128, (ch=4, 128)] bf16: 8 copies of the lora_A chunk per 16 cols,
       so mm1 psum rows 32j+k and 32j+16+k all hold tmp[k].
  mm1 per octet-pair P: psum p1 [128, (gp=2, 512)] fp32; copies
       p1[32j:32j+32] -> tmps[32j:32j+32] start at 32-aligned partitions
       (walrus requires 32-aligned compute-AP partition starts).
  tmps [128, (vs=16, j=4, g8=8, bl=8)] bf16: rows 32j..32j+15 hold tmp
       k-rows of var class j in its own column region, zero elsewhere
       (block-diagonal rhs); rows 32j+16..31 are duplicates that mm2
       multiplies by zero weight rows.
  lbw  [128, (q=4, g=16, o=128)] bf16: rows 32j+k = 256*lora_B[k, q*128+o,
       4g+j], rows 32j+16..31 = 0.
  mm2 per (q, g): lhsT = lbw slice [128, 128]; rhs = tmps [128, 256]
       contiguous -> psum [128 (o), (j, bl) = 256] fp32; 4 vars per matmul
       via the block-diagonal structure; DVE/ACT evacuate psum as fp8.
  out  [q=4][128, (g=16, j=4, b=64)] fp8 -> contiguous DMA per o-chunk q.

Workarounds for this container's walrus build:
  - every instruction may carry at most ONE semaphore wait: TileContext's
    tail drain is patched and a post-pass hoists excess waits onto NoOps.
  - compute-engine APs must start at 32-aligned partitions.
"""

import numpy as np
import ml_dtypes

import concourse.bass as bass
import concourse.mybir as mybir
import concourse.tile as tile
from concourse.bass_utils import run_bass_kernel_spmd

F32 = mybir.dt.float32
BF16 = mybir.dt.bfloat16
F8 = mybir.dt.float8e4
BFNP = ml_dtypes.bfloat16
F8NP = ml_dtypes.float8_e4m3
XS_SCALE = 64.0     # xs shipped as fp8(XS_SCALE * gating * x); tanh rescales
NX_SCALE = 256.0    # lora_B pre-scaled so fp8 nx' = NX_SCALE * nx

N_CORES = 8
B_FULL = 512
T = 512          # window length (= o dim)
V = 64           # n_var
R = 16           # low rank
B = B_FULL // N_CORES   # 64 batch elements per core
NCH = T // 128   # 4 t-chunks
NG8 = B // 8     # 8 batch-octets
NP = 4           # octet-pairs in phase 1
J = 4            # var classes packed per mm2 matmul
VS = V // J      # 16 var slots per class
XCOLS = B * NCH * V          # 16384
QCOL = XCOLS // 4            # 4096 (one octet-pair of xs columns)


def _patch_tile_tail():
    """Re-emit the kernel-tail Drain's semaphore waits as individual
    wait_ge instructions (walrus here rejects multi-wait instructions)."""
    if getattr(tile.TileContext, "_drain_patched", False):
        return

    def _drain_and_barrier(self, tick_clock, wait_clock):
        nc = self.nc
        from concourse.tile import ScopedClock

        drain_inst = nc.sync.drain()
        wait_clock.add_sem_waits(
            drain_inst.ins, ScopedClock({None: tick_clock.global_clock})
        )
        si = drain_inst.ins.sync_info
        waits = list(si.on_wait) if si is not None else []
        if len(waits) > 1:
            sems_by_name = {s.name: s for s in self.sems.allocated().values()}
            si.on_wait = []
            for w in waits:
                nc.sync.wait_ge(sems_by_name[w.ant_name], w.wait_value)
        nc.all_engine_barrier()
        popped = nc._tile_sem_poison_stack.pop()
        assert popped is self._sem_poison
        nc.clear_and_free_semaphores(list(self.sems.allocated().values()))
        nc.all_engine_barrier()

    tile.TileContext._drain_and_barrier = _drain_and_barrier
    tile.TileContext._drain_patched = True


def _split_multi_waits(nc, limit=1):
    """Hoist excess semaphore waits onto same-engine NoOps inserted just
    before the offending instruction (program order per engine preserves
    the wait-before-execute semantics)."""
    ctr = 0
    for f in nc.m.functions:
        for b in f.blocks:
            insts = list(b.instructions)
            out = []
            changed = False
            for inst in insts:
                si = inst.sync_info
                if si is not None:
                    waits = list(si.on_wait)
                    if len(waits) > limit:
                        for w in waits[:-limit]:
                            nop = mybir.InstNoOp(name=f"zzws_{ctr}")
                            ctr += 1
                            nop.engine = inst.engine
                            nop.sync_info = mybir.SyncInfo(
                                on_wait=[w], on_update=[]
                            )
                            out.append(nop)
                        si.on_wait = waits[-limit:]
                        changed = True
                out.append(inst)
            if changed:
                b.instructions = out
    return ctr


def build_program():
    _patch_tile_tail()
    nc = bass.Bass()

    xs_d = nc.dram_tensor("xs", [128, XCOLS], F8, kind="ExternalInput")
    a4_d = nc.dram_tensor("a4", [128, NCH * 128], BF16, kind="ExternalInput")
    lbw_d = nc.dram_tensor("lbw", [128, NCH * VS * 128], BF16, kind="ExternalInput")
    out_d = nc.dram_tensor("out", [NCH, 128, VS * J * B], F8, kind="ExternalOutput")

    with tile.TileContext(nc) as tc:
        with (
            tc.tile_pool(name="pers", bufs=1) as pers,
            tc.tile_pool(name="outp", bufs=2) as outp,
            tc.tile_pool(name="ps1", bufs=2, space="PSUM") as ps1,
            tc.tile_pool(name="ps2", bufs=3, space="PSUM") as ps2,
            tc.tile_pool(name="psw", bufs=1, space="PSUM") as psw,
        ):
            xs = pers.tile([128, XCOLS], F8)             # 16KB/partition
            x1 = pers.tile([128, XCOLS], BF16)
            a4 = pers.tile([128, NCH * 128], BF16)
            lbw = pers.tile([128, NCH * VS * 128], BF16)
            tmps = pers.tile([128, J * VS * B], BF16)    # [128, 4096]
            warm = psw.tile([128, 512], F32)             # HAM warm-up target
            junk = pers.tile([128, 512], BF16)           # never written: the
            # warm-up matmuls read it so they carry no input dependencies

            # --- input DMAs (xs quartered for overlap; first quarter split
            # into octets so the tanh chain starts as early as possible) ---
            nc.sync.dma_start(xs[:, 0:2048], xs_d[:, 0:2048])
            nc.sync.dma_start(xs[:, 2048:QCOL], xs_d[:, 2048:QCOL])
            nc.sync.dma_start(a4[:, :], a4_d[:, :])
            for P in range(1, NP):
                nc.sync.dma_start(
                    xs[:, P * QCOL : (P + 1) * QCOL],
                    xs_d[:, P * QCOL : (P + 1) * QCOL],
                )
            nc.sync.dma_start(lbw[:, :], lbw_d[:, :])

            # block-diagonal rhs: off-class regions of tmps must be zero
            nc.vector.memset(junk[:, :], 0.0)
            nc.vector.memset(tmps[:, :], 0.0)

            def warm_mms(n):
                # dummy matmuls: keep the PE's HAM activity window busy so
                # mm1/mm2 run at 2.4 GHz instead of the cold 1.2 GHz
                for _ in range(n):
                    nc.tensor.matmul(
                        warm[:, :], junk[:, 0:128], junk[:, :],
                        start=True, stop=True, skip_group_check=True,
                    )

            # --- phase 1: tanh + mm1 + tmp copies, per octet-pair P ---
            warm_mms(16)
            tv = tmps.rearrange("p (vs j g8 b) -> p g8 j vs b", vs=VS, j=J, g8=NG8)
            for P in range(NP):
                octet_split = P in (0, NP - 1)
                if octet_split:
                    # first pair: tanh starts on the smallest possible DMA;
                    # last pair: mm1/copies of octet 0 hide under octet 1's
                    # tanh so phase 2 starts sooner
                    for gp in range(2):
                        o8 = 2 * P + gp
                        nc.scalar.activation(
                            x1[:, o8 * 2048 : (o8 + 1) * 2048],
                            xs[:, o8 * 2048 : (o8 + 1) * 2048],
                            mybir.ActivationFunctionType.Tanh,
                            scale=1.0 / XS_SCALE,
                        )
                else:
                    nc.scalar.activation(
                        x1[:, P * QCOL : (P + 1) * QCOL],
                        xs[:, P * QCOL : (P + 1) * QCOL],
                        mybir.ActivationFunctionType.Tanh,
                        scale=1.0 / XS_SCALE,
                    )
                p1 = ps1.tile([128, 2 * 512], F32)       # 2 banks
                p1v = p1.rearrange("p (gp j vs b) -> p gp j vs b", gp=2, j=J, vs=VS)
                for gp in range(2):
                    g8 = 2 * P + gp
                    for ch in range(NCH):
                        nc.tensor.matmul(
                            p1[:, gp * 512 : (gp + 1) * 512],
                            a4[:, ch * 128 : (ch + 1) * 128],
                            x1[:, (g8 * NCH + ch) * 512 : (g8 * NCH + ch + 1) * 512],
                            start=(ch == 0),
                            stop=(ch == NCH - 1),
                        )
                    if P == NP - 1:
                        # per-octet copies (different psum bank than the
                        # in-flight gp=1 matmuls)
                        for j in range(J):
                            dst = tv[32 * j : 32 * j + 32, g8 : g8 + 1, j]
                            srcv = p1v[32 * j : 32 * j + 32, gp : gp + 1, j]
                            if gp == 1 and j % 2 == 1:
                                nc.scalar.copy(dst, srcv)
                            else:
                                nc.vector.tensor_copy(dst, srcv)
                if P < NP - 1:
                    for j in range(J):
                        nc.vector.tensor_copy(
                            tv[32 * j : 32 * j + 32, 2 * P : 2 * P + 2, j],
                            p1v[32 * j : 32 * j + 32, :, j],
                        )
                warm_mms(0 if P < NP - 1 else 6)

            # --- phase 2: mm2 + evac + out DMA, per o-chunk q ---
            tmpv = tmps.rearrange("p (g c) -> p g c", g=VS)
            for q in range(NCH):
                out_t = outp.tile([128, VS * J * B], F8)
                for g2 in range(8):
                    pb = ps2.tile([128, 2 * J * B], F32)    # 1 bank
                    for gg in range(2):
                        g = 2 * g2 + gg
                        nc.tensor.matmul(
                            pb[:, gg * 256 : (gg + 1) * 256],
                            lbw[:, (q * VS + g) * 128 : (q * VS + g + 1) * 128],
                            tmpv[:, g],
                            start=True,
                            stop=True,
                        )
                    # evac psum -> fp8 out tile, alternating DVE / ACT
                    dst = out_t[:, g2 * 512 : (g2 + 1) * 512]
                    if g2 % 2 == 0:
                        nc.vector.tensor_copy(dst, pb[:, :])
                    else:
                        nc.scalar.copy(dst, pb[:, :])
                    if g2 == 3:
                        nc.sync.dma_start(
                            out_d[q, :, 0:2048], out_t[:, 0:2048]
                        )
                nc.sync.dma_start(out_d[q, :, 2048:], out_t[:, 2048:])

    n_split = _split_multi_waits(nc)
    print(f"[kernel] wait-split nops inserted: {n_split}")
    return nc


_PROGRAM = None


def _get_program():
    global _PROGRAM
    if _PROGRAM is None:
        _PROGRAM = build_program()
    return _PROGRAM


def _host_prep(x, gating, lora_A, lora_B):
    # xs per core: [128, g8, ch, j, vs, bl] = gating[v] * x[b, t, v]
    # x: [512, 512, 64] -> [c, g8, bl, ch, p, vs, j]
    g2 = gating.reshape(VS, J)  # v = 4*vs + j
    xr = x.reshape(N_CORES, NG8, 8, NCH, 128, VS, J) * (XS_SCALE * g2)
    np.clip(xr, -240.0, 240.0, out=xr)
    xs_all = np.ascontiguousarray(
        xr.transpose(0, 4, 1, 3, 6, 5, 2).astype(F8NP)
    ).reshape(N_CORES, 128, XCOLS)

    # a4: [p, ch, 16*cblk + k] = lora_A[ch*128+p, k], 8 copies over cblk
    ap = lora_A.reshape(NCH, 128, R).transpose(1, 0, 2)  # [p, ch, k]
    a4 = np.ascontiguousarray(
        np.broadcast_to(ap[:, :, None, :], (128, NCH, 8, R)).astype(BFNP)
    ).reshape(128, NCH * 128)

    # lbw: rows 32j+k = B[k, q*128+o, 4g+j]; rows 32j+16.. = 0
    lb = (NX_SCALE * lora_B).reshape(R, NCH, 128, VS, J).transpose(4, 0, 1, 3, 2)  # [j,k,q,g,o]
    lbw = np.zeros((J, 32, NCH, VS, 128), dtype=BFNP)
    lbw[:, :R] = lb.astype(BFNP)
    lbw = lbw.reshape(128, NCH * VS * 128)
    return xs_all, a4, lbw


def kernel(x, gating, bias, lora_A, lora_B):
    x = np.ascontiguousarray(np.asarray(x, dtype=np.float32)).reshape(B_FULL, T, V)
    gating = np.asarray(gating, dtype=np.float32)
    bias = np.asarray(bias, dtype=np.float32)
    lora_A = np.ascontiguousarray(np.asarray(lora_A, dtype=np.float32))
    lora_B = np.ascontiguousarray(np.asarray(lora_B, dtype=np.float32))

    xs_all, a4, lbw = _host_prep(x, gating, lora_A, lora_B)

    nc = _get_program()
    in_maps = []
    for c in range(N_CORES):
        in_maps.append({"xs": xs_all[c], "a4": a4, "lbw": lbw})
    res = run_bass_kernel_spmd(nc, in_maps, core_ids=list(range(N_CORES)))

    out = np.empty((B_FULL, T, V), dtype=np.float32)
    xb = x.reshape(N_CORES, B, T, V)
    for c in range(N_CORES):
        # out_d [q, p, (g, j, b)] -> nx[b, (q, p), (vs=g, j)]
        nx = (
            res.results[c]["out"]
            .reshape(NCH, 128, VS, J, B)
            .transpose(4, 0, 1, 2, 3)
            .astype(np.float32)
            .reshape(B, T, V)
        )
        out[c * B : (c + 1) * B] = xb[c] + nx * (1.0 / NX_SCALE) + bias
    return out.reshape(B_FULL, T, V, 1)


# revision 31
# speedup vs baseline: 1.0794x; 1.0016x over previous
"""Trainium2 Bass kernel for per-variable gated LoRA mixer (dense_mlp).

Math (reference):
    xr  = x.reshape(b, t, v)                  # b=512, t=512, v=64
    x1  = tanh(gating * xr)
    tmp = einsum('biv,ik->bkv', x1, lora_A)   # r=16
    nx  = einsum('bkv,kov->bov', tmp, lora_B)
    out = xr + nx + bias

Split of work (data-parallel over batch, 64 per core, params replicated):
  HOST (numpy): scale/transpose inputs into matmul-ready SBUF images
    (xs = fp8(64*gating*x), lora_A copies, 256*lora_B in the block layout),
    and the final fp32 combine out = x + nx/256 + bias.
  DEVICE (per core): x1 = tanh(xs/64) on ACT (scale folded into the
    activation); mm1 (contract t) and mm2 (contract r) on the PE in
    bf16/fp8 with fp32 psum; psum evacuation split DVE/ACT; fp8 output.

Device layouts (b = 64 local batch, v = 4*vs + j):
  xs   [128, (g8=8, ch=4, j=4, vs=16, bl=8)] fp8: partition p = t within
       chunk ch (t = ch*128+p); local batch = g8*8+bl.
  a4   [128, (ch=4, 128)] bf16: 8 copies of the lora_A chunk per 16 cols,
       so mm1 psum rows 32j+k and 32j+16+k all hold tmp[k].
  mm1 per octet-pair P: psum p1 [128, (gp=2, 512)] fp32; copies
       p1[32j:32j+32] -> tmps[32j:32j+32] start at 32-aligned partitions
       (walrus requires 32-aligned compute-AP partition starts).
  tmps [128, (vs=16, j=4, g8=8, bl=8)] bf16: rows 32j..32j+15 hold tmp
       k-rows of var class j in its own column region, zero elsewhere
       (block-diagonal rhs); rows 32j+16..31 are duplicates that mm2
       multiplies by zero weight rows.
  lbw  [128, (q=4, g=16, o=128)] bf16: rows 32j+k = 256*lora_B[k, q*128+o,
       4g+j], rows 32j+16..31 = 0.
  mm2 per (q, g): lhsT = lbw slice [128, 128]; rhs = tmps [128, 256]
       contiguous -> psum [128 (o), (j, bl) = 256] fp32; 4 vars per matmul
       via the block-diagonal structure; DVE/ACT evacuate psum as fp8.
  out  [q=4][128, (g=16, j=4, b=64)] fp8 -> contiguous DMA per o-chunk q.

Workarounds for this container's walrus build:
  - every instruction may carry at most ONE semaphore wait: TileContext's
    tail drain is patched and a post-pass hoists excess waits onto NoOps.
  - compute-engine APs must start at 32-aligned partitions.
"""

import numpy as np
import ml_dtypes

import concourse.bass as bass
import concourse.mybir as mybir
import concourse.tile as tile
from concourse.bass_utils import run_bass_kernel_spmd

F32 = mybir.dt.float32
BF16 = mybir.dt.bfloat16
F8 = mybir.dt.float8e4
BFNP = ml_dtypes.bfloat16
F8NP = ml_dtypes.float8_e4m3
XS_SCALE = 64.0     # xs shipped as fp8(XS_SCALE * gating * x); tanh rescales
NX_SCALE = 256.0    # lora_B pre-scaled so fp8 nx' = NX_SCALE * nx

N_CORES = 8
B_FULL = 512
T = 512          # window length (= o dim)
V = 64           # n_var
R = 16           # low rank
B = B_FULL // N_CORES   # 64 batch elements per core
NCH = T // 128   # 4 t-chunks
NG8 = B // 8     # 8 batch-octets
NP = 4           # octet-pairs in phase 1
J = 4            # var classes packed per mm2 matmul
VS = V // J      # 16 var slots per class
XCOLS = B * NCH * V          # 16384
QCOL = XCOLS // 4            # 4096 (one octet-pair of xs columns)


def _patch_tile_tail():
    """Re-emit the kernel-tail Drain's semaphore waits as individual
    wait_ge instructions (walrus here rejects multi-wait instructions)."""
    if getattr(tile.TileContext, "_drain_patched", False):
        return

    def _drain_and_barrier(self, tick_clock, wait_clock):
        nc = self.nc
        from concourse.tile import ScopedClock

        drain_inst = nc.sync.drain()
        wait_clock.add_sem_waits(
            drain_inst.ins, ScopedClock({None: tick_clock.global_clock})
        )
        si = drain_inst.ins.sync_info
        waits = list(si.on_wait) if si is not None else []
        if len(waits) > 1:
            sems_by_name = {s.name: s for s in self.sems.allocated().values()}
            si.on_wait = []
            for w in waits:
                nc.sync.wait_ge(sems_by_name[w.ant_name], w.wait_value)
        nc.all_engine_barrier()
        popped = nc._tile_sem_poison_stack.pop()
        assert popped is self._sem_poison
        nc.clear_and_free_semaphores(list(self.sems.allocated().values()))
        nc.all_engine_barrier()

    tile.TileContext._drain_and_barrier = _drain_and_barrier
    tile.TileContext._drain_patched = True


def _split_multi_waits(nc, limit=1):
    """Hoist excess semaphore waits onto same-engine NoOps inserted just
    before the offending instruction (program order per engine preserves
    the wait-before-execute semantics)."""
    ctr = 0
    for f in nc.m.functions:
        for b in f.blocks:
            insts = list(b.instructions)
            out = []
            changed = False
            for inst in insts:
                si = inst.sync_info
                if si is not None:
                    waits = list(si.on_wait)
                    if len(waits) > limit:
                        for w in waits[:-limit]:
                            nop = mybir.InstNoOp(name=f"zzws_{ctr}")
                            ctr += 1
                            nop.engine = inst.engine
                            nop.sync_info = mybir.SyncInfo(
                                on_wait=[w], on_update=[]
                            )
                            out.append(nop)
                        si.on_wait = waits[-limit:]
                        changed = True
                out.append(inst)
            if changed:
                b.instructions = out
    return ctr


def build_program():
    _patch_tile_tail()
    nc = bass.Bass()

    xs_d = nc.dram_tensor("xs", [128, XCOLS], F8, kind="ExternalInput")
    a4_d = nc.dram_tensor("a4", [128, NCH * 128], BF16, kind="ExternalInput")
    lbw_d = nc.dram_tensor("lbw", [128, NCH * VS * 128], BF16, kind="ExternalInput")
    out_d = nc.dram_tensor("out", [NCH, 128, VS * J * B], F8, kind="ExternalOutput")

    with tile.TileContext(nc) as tc:
        with (
            tc.tile_pool(name="pers", bufs=1) as pers,
            tc.tile_pool(name="outp", bufs=2) as outp,
            tc.tile_pool(name="ps1", bufs=2, space="PSUM") as ps1,
            tc.tile_pool(name="ps2", bufs=3, space="PSUM") as ps2,
            tc.tile_pool(name="psw", bufs=1, space="PSUM") as psw,
        ):
            xs = pers.tile([128, XCOLS], F8)             # 16KB/partition
            x1 = pers.tile([128, XCOLS], BF16)
            a4 = pers.tile([128, NCH * 128], BF16)
            lbw = pers.tile([128, NCH * VS * 128], BF16)
            tmps = pers.tile([128, J * VS * B], BF16)    # [128, 4096]
            warm = psw.tile([128, 512], F32)             # HAM warm-up target
            junk = pers.tile([128, 512], BF16)           # never written: the
            # warm-up matmuls read it so they carry no input dependencies

            # --- input DMAs (xs quartered for overlap; first quarter split
            # into octets so the tanh chain starts as early as possible) ---
            nc.sync.dma_start(xs[:, 0:2048], xs_d[:, 0:2048])
            nc.sync.dma_start(xs[:, 2048:QCOL], xs_d[:, 2048:QCOL])
            nc.sync.dma_start(a4[:, :], a4_d[:, :])
            for P in range(1, NP):
                nc.sync.dma_start(
                    xs[:, P * QCOL : (P + 1) * QCOL],
                    xs_d[:, P * QCOL : (P + 1) * QCOL],
                )
            nc.sync.dma_start(lbw[:, :], lbw_d[:, :])

            # block-diagonal rhs: off-class regions of tmps must be zero
            nc.vector.memset(junk[:, :], 0.0)
            nc.vector.memset(tmps[:, :], 0.0)

            def warm_mms(n):
                # dummy matmuls: keep the PE's HAM activity window busy so
                # mm1/mm2 run at 2.4 GHz instead of the cold 1.2 GHz
                for _ in range(n):
                    nc.tensor.matmul(
                        warm[:, :], junk[:, 0:128], junk[:, :],
                        start=True, stop=True, skip_group_check=True,
                    )

            # --- phase 1: tanh + mm1 + tmp copies, per octet-pair P ---
            warm_mms(10)
            tv = tmps.rearrange("p (vs j g8 b) -> p g8 j vs b", vs=VS, j=J, g8=NG8)
            for P in range(NP):
                octet_split = P in (0, NP - 1)
                if octet_split:
                    # first pair: tanh starts on the smallest possible DMA;
                    # last pair: mm1/copies of octet 0 hide under octet 1's
                    # tanh so phase 2 starts sooner
                    for gp in range(2):
                        o8 = 2 * P + gp
                        nc.scalar.activation(
                            x1[:, o8 * 2048 : (o8 + 1) * 2048],
                            xs[:, o8 * 2048 : (o8 + 1) * 2048],
                            mybir.ActivationFunctionType.Tanh,
                            scale=1.0 / XS_SCALE,
                        )
                else:
                    nc.scalar.activation(
                        x1[:, P * QCOL : (P + 1) * QCOL],
                        xs[:, P * QCOL : (P + 1) * QCOL],
                        mybir.ActivationFunctionType.Tanh,
                        scale=1.0 / XS_SCALE,
                    )
                p1 = ps1.tile([128, 2 * 512], F32)       # 2 banks
                p1v = p1.rearrange("p (gp j vs b) -> p gp j vs b", gp=2, j=J, vs=VS)
                for gp in range(2):
                    g8 = 2 * P + gp
                    for ch in range(NCH):
                        nc.tensor.matmul(
                            p1[:, gp * 512 : (gp + 1) * 512],
                            a4[:, ch * 128 : (ch + 1) * 128],
                            x1[:, (g8 * NCH + ch) * 512 : (g8 * NCH + ch + 1) * 512],
                            start=(ch == 0),
                            stop=(ch == NCH - 1),
                        )
                    if P == NP - 1:
                        # per-octet copies (different psum bank than the
                        # in-flight gp=1 matmuls)
                        for j in range(J):
                            dst = tv[32 * j : 32 * j + 32, g8 : g8 + 1, j]
                            srcv = p1v[32 * j : 32 * j + 32, gp : gp + 1, j]
                            if gp == 1 and j % 2 == 1:
                                nc.scalar.copy(dst, srcv)
                            else:
                                nc.vector.tensor_copy(dst, srcv)
                if P < NP - 1:
                    for j in range(J):
                        nc.vector.tensor_copy(
                            tv[32 * j : 32 * j + 32, 2 * P : 2 * P + 2, j],
                            p1v[32 * j : 32 * j + 32, :, j],
                        )
                warm_mms(0 if P < NP - 1 else 6)

            # --- phase 2: mm2 + evac + out DMA, per o-chunk q ---
            tmpv = tmps.rearrange("p (g c) -> p g c", g=VS)
            for q in range(NCH):
                out_t = outp.tile([128, VS * J * B], F8)
                for g2 in range(8):
                    pb = ps2.tile([128, 2 * J * B], F32)    # 1 bank
                    for gg in range(2):
                        g = 2 * g2 + gg
                        nc.tensor.matmul(
                            pb[:, gg * 256 : (gg + 1) * 256],
                            lbw[:, (q * VS + g) * 128 : (q * VS + g + 1) * 128],
                            tmpv[:, g],
                            start=True,
                            stop=True,
                        )
                    # evac psum -> fp8 out tile, alternating DVE / ACT
                    dst = out_t[:, g2 * 512 : (g2 + 1) * 512]
                    if g2 % 2 == 0:
                        nc.vector.tensor_copy(dst, pb[:, :])
                    else:
                        nc.scalar.copy(dst, pb[:, :])
                    if g2 == 3:
                        nc.sync.dma_start(
                            out_d[q, :, 0:2048], out_t[:, 0:2048]
                        )
                nc.sync.dma_start(out_d[q, :, 2048:], out_t[:, 2048:])

    n_split = _split_multi_waits(nc)
    print(f"[kernel] wait-split nops inserted: {n_split}")
    return nc


_PROGRAM = None


def _get_program():
    global _PROGRAM
    if _PROGRAM is None:
        _PROGRAM = build_program()
    return _PROGRAM


def _host_prep(x, gating, lora_A, lora_B):
    # xs per core: [128, g8, ch, j, vs, bl] = gating[v] * x[b, t, v]
    # x: [512, 512, 64] -> [c, g8, bl, ch, p, vs, j]
    g2 = gating.reshape(VS, J)  # v = 4*vs + j
    xr = x.reshape(N_CORES, NG8, 8, NCH, 128, VS, J) * (XS_SCALE * g2)
    np.clip(xr, -240.0, 240.0, out=xr)
    xs_all = np.ascontiguousarray(
        xr.transpose(0, 4, 1, 3, 6, 5, 2).astype(F8NP)
    ).reshape(N_CORES, 128, XCOLS)

    # a4: [p, ch, 16*cblk + k] = lora_A[ch*128+p, k], 8 copies over cblk
    ap = lora_A.reshape(NCH, 128, R).transpose(1, 0, 2)  # [p, ch, k]
    a4 = np.ascontiguousarray(
        np.broadcast_to(ap[:, :, None, :], (128, NCH, 8, R)).astype(BFNP)
    ).reshape(128, NCH * 128)

    # lbw: rows 32j+k = B[k, q*128+o, 4g+j]; rows 32j+16.. = 0
    lb = (NX_SCALE * lora_B).reshape(R, NCH, 128, VS, J).transpose(4, 0, 1, 3, 2)  # [j,k,q,g,o]
    lbw = np.zeros((J, 32, NCH, VS, 128), dtype=BFNP)
    lbw[:, :R] = lb.astype(BFNP)
    lbw = lbw.reshape(128, NCH * VS * 128)
    return xs_all, a4, lbw


def kernel(x, gating, bias, lora_A, lora_B):
    x = np.ascontiguousarray(np.asarray(x, dtype=np.float32)).reshape(B_FULL, T, V)
    gating = np.asarray(gating, dtype=np.float32)
    bias = np.asarray(bias, dtype=np.float32)
    lora_A = np.ascontiguousarray(np.asarray(lora_A, dtype=np.float32))
    lora_B = np.ascontiguousarray(np.asarray(lora_B, dtype=np.float32))

    xs_all, a4, lbw = _host_prep(x, gating, lora_A, lora_B)

    nc = _get_program()
    in_maps = []
    for c in range(N_CORES):
        in_maps.append({"xs": xs_all[c], "a4": a4, "lbw": lbw})
    res = run_bass_kernel_spmd(nc, in_maps, core_ids=list(range(N_CORES)))

    out = np.empty((B_FULL, T, V), dtype=np.float32)
    xb = x.reshape(N_CORES, B, T, V)
    for c in range(N_CORES):
        # out_d [q, p, (g, j, b)] -> nx[b, (q, p), (vs=g, j)]
        nx = (
            res.results[c]["out"]
            .reshape(NCH, 128, VS, J, B)
            .transpose(4, 0, 1, 2, 3)
            .astype(np.float32)
            .reshape(B, T, V)
        )
        out[c * B : (c + 1) * B] = xb[c] + nx * (1.0 / NX_SCALE) + bias
    return out.reshape(B_FULL, T, V, 1)
